# revision 40
# baseline (speedup 1.0000x reference)
"""CQAttention Trainium2 kernel.

Math (per batch b):
  S = (C*w3) @ Q^T + (C@w1)[:,None] + (Q@w2)[None,:] (+bias, dropped: softmax-invariant)
  Sq = softmax over q of qmask-masked S ; Sc = softmax over c of cmask-masked S
  A = Sq@Q ; Bm = Sq @ (Sc^T @ C) ; out = [C | A | C*A | C*Bm]

Device algorithm (no max-subtraction: |S| < 1 so exp is safe; masks become
additive -1e30 terms):
  ST   = (Q*w3) @ C^T                        [q, c]  (PE; stationary QT3)
  E_q  = exp(ST + (rq + qneg)[q])            [q, c]  row-masked (ACT bias)
  STT  = C @ [(Q*w3)^T | w1 w1]              [c, q+2] (PE; stationary CT tile;
         cols 128:130 accumulate rc = C@w1 for free)
  Xg   = exp(STT[:, :128] + (rc + cneg)[c])  [c, q]  col-mask+rc via ACT bias
  T1s  = (Xg^T @ [C|1]) normalized           [q, d]  == Sc^T @ C
  psA  = E_q^T @ [Q|1]                       [c, d+2] unnormalized A | rowsum
  psB  = E_q^T @ T1s                         [c, d]   unnormalized Bm
  A = psA * rr ; CA = C * A ; CBm = C * psB * rr      (rr = 1/rowsum)

Layout: c-row mapping c = 256g + 2p + j (partition p owns 2 consecutive HBM
rows per quarter-batch g), so each quarter-batch output store is one DMA with
8KB contiguous per partition, and C is DMA'd directly into the output tile
(no engine copy, no extra store).

Sharding: data-parallel over batch, 4 batches per core on 8 cores.
"""

import numpy as np

NEG_INF = -1e30
B_FULL, LC, LQ, D = 32, 1024, 128, 256
N_CORES = 8
NB = B_FULL // N_CORES  # batches per core
KC = LC // 128  # c-tiles per batch (8)

_CACHE = {}


def _build_nc():
    import concourse.bacc as bacc
    import concourse.mybir as mybir
    from concourse import tile
    from concourse.masks import make_identity

    fp32 = mybir.dt.float32
    fp32r = mybir.dt.float32r
    mmdt = fp32r
    MULT = mybir.AluOpType.mult
    ADD = mybir.AluOpType.add
    EXP = mybir.ActivationFunctionType.Exp

    nc = bacc.Bacc("TRN2", target_bir_lowering=False, debug=False)

    C_d = nc.dram_tensor("C", [NB, LC, D], fp32, kind="ExternalInput")
    qt3_d = nc.dram_tensor("QT3a", [NB, 128, 2, 128], fp32r, kind="ExternalInput")
    qr_d = nc.dram_tensor("Qr", [NB, 128, D + 2], fp32r, kind="ExternalInput")
    cb_d = nc.dram_tensor("cb", [NB, 128, KC], fp32, kind="ExternalInput")
    qb_d = nc.dram_tensor("qb", [NB, 128, 1], fp32, kind="ExternalInput")
    out_d = nc.dram_tensor("out", [NB, LC, 4 * D], fp32, kind="ExternalOutput")

    with tile.TileContext(nc) as tc:
        with (
            tc.tile_pool(name="const", bufs=1) as const,
            tc.tile_pool(name="crpool", bufs=2) as p_cr,
            tc.tile_pool(name="qpool", bufs=NB) as p_q,
            tc.tile_pool(name="mpool", bufs=NB) as p_m,
            tc.tile_pool(name="ctpool", bufs=3) as p_ct,
            tc.tile_pool(name="qtpool", bufs=2) as p_qt,
            tc.tile_pool(name="epool", bufs=3) as p_e,
            tc.tile_pool(name="xgpool", bufs=3) as p_xg,
            tc.tile_pool(name="opool", bufs=12) as p_o,
            tc.tile_pool(name="smpool", bufs=8) as p_sm,
            tc.tile_pool(name="flex", bufs=2, space="PSUM") as ps_flex,
            tc.tile_pool(name="psstt", bufs=2, space="PSUM") as ps_stt,
            tc.tile_pool(name="pst1", bufs=1, space="PSUM") as ps_t1,
            tc.tile_pool(name="psab", bufs=3, space="PSUM") as ps_ab,
        ):
            ident = const.tile([128, 128], fp32)
            make_identity(nc, ident)
            # warm up the ACT exp table while input DMAs run
            wtmp = const.tile([128, 1], fp32, tag="wtmp")
            nc.scalar.activation(wtmp, ident[:, 0:1], EXP)

            # C1r slots: fp32r copies of C for the T1 matmul moving operand.
            # The two ones-pad columns are written once per slot and persist.
            cr_slots = []
            for _ in range(2):
                cr = p_cr.tile([128, KC, D + 2], mmdt, tag="cr")
                nc.vector.memset(cr.bitcast(fp32)[:, :, D : D + 2], 1.0)
                cr_slots.append(cr)

            # ---- hoisted input loads for all batches ----
            osbs, Q1s, cnegs, qnegs = [], [], [], []
            for b in range(NB):
                # output accumulation tiles [C | A | C*A | C*Bm] per half-batch;
                # C lands in cols 0:D straight from HBM (c = 512h+4p+j mapping).
                osb = []
                for g in range(4):
                    o = p_o.tile([128, 2, 4 * D], fp32, tag="osb")
                    nc.sync.dma_start(
                        o[:, :, 0:D],
                        C_d.ap()[b, g * 256 : (g + 1) * 256].rearrange(
                            "(p j) d -> p j d", p=128
                        ),
                    )
                    osb.append(o)
                QT3 = p_q.tile([128, 2, 128], mmdt, tag="qt3")
                nc.sync.dma_start(QT3, qt3_d.ap()[b])
                Q1r = p_q.tile([128, D + 2], mmdt, tag="qr")
                nc.sync.dma_start(Q1r, qr_d.ap()[b])
                cb = p_m.tile([128, KC], fp32, tag="cb")
                nc.sync.dma_start(cb, cb_d.ap()[b])
                qb = p_m.tile([128, 1], fp32, tag="qb")
                nc.sync.dma_start(qb, qb_d.ap()[b])
                osbs.append(osb)
                Q1s.append((QT3, Q1r))
                cnegs.append(cb)
                qnegs.append(qb)

            for b in range(NB):
                osb, (QT3, Q1r), cb, bias_q = osbs[b], Q1s[b], cnegs[b], qnegs[b]

                def Cb(t):
                    return osb[t // 2][:, t % 2, 0:D]

                # rounded fp32r copies for matmul moving operands
                C1r = cr_slots[b % 2]
                for g in range(4):
                    nc.vector.tensor_copy(
                        C1r[:, g * 2 : (g + 1) * 2, 0:D], osb[g][:, :, 0:D]
                    )

                # ---- CT (transpose C): 4 transposes per PSUM bank, 1 copy ----
                CT = p_ct.tile([128, 2, LC], mmdt, tag="ct")
                for dk in range(2):
                    for h in range(2):
                        pt = ps_flex.tile([128, 512], fp32, tag="flex")
                        for jj in range(4):
                            j = h * 4 + jj
                            nc.tensor.transpose(
                                pt[:, jj * 128 : (jj + 1) * 128],
                                Cb(j)[:, dk * 128 : (dk + 1) * 128],
                                ident,
                            )
                        dst = CT[:, dk, h * 512 : (h + 1) * 512]
                        nc.vector.tensor_copy(dst, pt)

                # ---- ST = (Q*w3) @ C^T, E_q = exp(ST + bias_q) ----
                E_q = p_e.tile([128, LC], mmdt, tag="eq")
                for h in range(2):
                    st = ps_flex.tile([128, 512], fp32, tag="flex")
                    for dk in range(2):
                        nc.tensor.matmul(
                            st,
                            QT3[:, dk],
                            CT[:, dk, h * 512 : (h + 1) * 512],
                            start=(dk == 0),
                            stop=(dk == 1),
                        )
                    nc.scalar.activation(
                        E_q[:, h * 512 : (h + 1) * 512], st, EXP, bias=bias_q
                    )

                # ---- STT tiles: [c, q | rc] ; Xg = exp(STT + rc + cneg) ----
                Xg = p_xg.tile([128, KC, 128], mmdt, tag="xg")
                for j in range(KC):
                    stt = ps_stt.tile([128, 128], fp32, tag="stt")
                    for dk in range(2):
                        nc.tensor.matmul(
                            stt,
                            CT[:, dk, j * 128 : (j + 1) * 128],
                            QT3[:, dk],
                            start=(dk == 0),
                            stop=(dk == 1),
                        )
                    nc.scalar.activation(
                        Xg[:, j], stt, EXP, bias=cb[:, j : j + 1]
                    )

                # ---- T1s = (Sc^T C) = (Xg^T @ [C|1]) normalized ----
                t1 = ps_t1.tile([128, D + 2], fp32, tag="t1")
                for j in range(KC):
                    nc.tensor.matmul(
                        t1,
                        Xg[:, j],
                        C1r[:, j],
                        start=(j == 0),
                        stop=(j == KC - 1),
                    )
                recipT = p_sm.tile([128, 1], fp32, tag="recipT")
                nc.vector.reciprocal(recipT, t1[:, D : D + 1])
                T1s = p_sm.tile([128, D], mmdt, tag="t1s")
                nc.vector.tensor_scalar_mul(T1s, t1[:, 0:D], recipT)

                # ---- per c-tile: A / CA / CBm into osb segments ----
                for t in range(KC):
                    h, j = t // 2, t % 2
                    o = osb[h]
                    eq_j = E_q[:, t * 128 : (t + 1) * 128]
                    psA = ps_ab.tile([128, D + 2], fp32, tag="ab")
                    nc.tensor.matmul(psA, eq_j, Q1r[:], start=True, stop=True)
                    psBt = ps_ab.tile([128, D + 2], fp32, tag="ab")
                    psB = psBt[:, 0:D]
                    nc.tensor.matmul(psB, eq_j, T1s[:], start=True, stop=True)

                    rr = p_sm.tile([128, 1], fp32, tag="rr")
                    nc.vector.reciprocal(rr, psA[:, D : D + 1])

                    # A = psA * rr  (ACT, per-partition scale)
                    nc.scalar.mul(o[:, j, D : 2 * D], psA[:, 0:D], rr)
                    # CA = C * A  (GPSIMD, reads the extracted A)
                    nc.gpsimd.tensor_mul(
                        o[:, j, 2 * D : 3 * D],
                        o[:, j, 0:D],
                        o[:, j, D : 2 * D],
                    )
                    # CBm = (psB * rr) * C  (DVE fused)
                    nc.vector.scalar_tensor_tensor(
                        o[:, j, 3 * D : 4 * D],
                        psB,
                        rr,
                        o[:, j, 0:D],
                        MULT,
                        MULT,
                    )
                    # store each quarter as soon as its 2 c-tiles are done:
                    # 1MB DMA, 8KB contiguous per partition
                    if j == 1:
                        nc.sync.dma_start(
                            out_d.ap()[b, h * 256 : (h + 1) * 256].rearrange(
                                "(p j) n -> p j n", p=128
                            ),
                            osb[h],
                        )

    nc.compile()
    return nc


def _get_nc():
    if "nc" not in _CACHE:
        _CACHE["nc"] = _build_nc()
    return _CACHE["nc"]


def _make_in_maps(C, Q, cmask, qmask, Wo_w):
    """Host preprocessing: biases (rq+qneg, rc+cneg), Q^T*w3, [Q|1] fp32r."""
    C = np.ascontiguousarray(C, dtype=np.float32)
    Q = np.ascontiguousarray(Q, dtype=np.float32)
    w1 = Wo_w[0:D].astype(np.float32)
    w2 = Wo_w[D : 2 * D].astype(np.float32)
    w3 = Wo_w[2 * D :].astype(np.float32)
    rc = C @ w1  # [B, LC]
    rq = Q @ w2  # [B, LQ]
    cneg = rc + (1.0 - cmask.astype(np.float32)) * NEG_INF
    qneg = rq + (1.0 - qmask.astype(np.float32)) * NEG_INF
    cneg = cneg.astype(np.float32)
    qneg = qneg.astype(np.float32)
    # c = 256g + 2p + j mapping -> [b, p, (g j)]
    cneg = np.ascontiguousarray(
        cneg.reshape(B_FULL, 4, 128, 2).transpose(0, 2, 1, 3).reshape(B_FULL, 128, KC)
    )
    qneg = np.ascontiguousarray(qneg.reshape(B_FULL, 128, 1))
    # QT3a[b, p, dk, q] = Q[b, q, dk*128+p] * w3[dk*128+p]
    QT3a = np.ascontiguousarray(
        (Q * w3).transpose(0, 2, 1).reshape(B_FULL, 2, 128, LQ).transpose(0, 2, 1, 3)
    )
    Qr = np.ones((B_FULL, 128, D + 2), dtype=np.float32)
    Qr[:, :, 0:D] = Q
    in_maps = []
    for i in range(N_CORES):
        sl = slice(i * NB, (i + 1) * NB)
        in_maps.append(
            {
                "C": np.ascontiguousarray(C[sl]),
                "QT3a": np.ascontiguousarray(QT3a[sl]),
                "Qr": np.ascontiguousarray(Qr[sl]),
                "cb": np.ascontiguousarray(cneg[sl]),
                "qb": np.ascontiguousarray(qneg[sl]),
            }
        )
    return in_maps


def kernel(C, Q, cmask, qmask, Wo_w, Wo_b):
    from concourse.bass_utils import run_bass_kernel_spmd

    nc = _get_nc()
    in_maps = _make_in_maps(C, Q, cmask, qmask, Wo_w)
    res = run_bass_kernel_spmd(nc, in_maps, core_ids=list(range(N_CORES)))
    out = np.concatenate([res.results[i]["out"] for i in range(N_CORES)], axis=0)
    return out


# revision 41
# speedup vs baseline: 1.0894x; 1.0894x over previous
"""CQAttention Trainium2 kernel.

Math (per batch b):
  S = (C*w3) @ Q^T + (C@w1)[:,None] + (Q@w2)[None,:] (+bias, dropped: softmax-invariant)
  Sq = softmax over q of qmask-masked S ; Sc = softmax over c of cmask-masked S
  A = Sq@Q ; Bm = Sq @ (Sc^T @ C) ; out = [C | A | C*A | C*Bm]

Device algorithm (no max-subtraction: |S| < 1 so exp is safe; masks become
additive -1e30 terms):
  ST   = (Q*w3) @ C^T                        [q, c]  (PE; stationary QT3)
  E_q  = exp(ST + (rq + qneg)[q])            [q, c]  row-masked (ACT bias)
  STT  = C @ [(Q*w3)^T | w1 w1]              [c, q+2] (PE; stationary CT tile;
         cols 128:130 accumulate rc = C@w1 for free)
  Xg   = exp(STT[:, :128] + (rc + cneg)[c])  [c, q]  col-mask+rc via ACT bias
  T1s  = (Xg^T @ [C|1]) normalized           [q, d]  == Sc^T @ C
  psA  = E_q^T @ [Q|1]                       [c, d+2] unnormalized A | rowsum
  psB  = E_q^T @ T1s                         [c, d]   unnormalized Bm
  A = psA * rr ; CA = C * A ; CBm = C * psB * rr      (rr = 1/rowsum)

Layout: c-row mapping c = 256g + 2p + j (partition p owns 2 consecutive HBM
rows per quarter-batch g), so each quarter-batch output store is one DMA with
8KB contiguous per partition, and C is DMA'd directly into the output tile
(no engine copy, no extra store).

Sharding: data-parallel over batch, 4 batches per core on 8 cores.
"""

import numpy as np

NEG_INF = -1e30
B_FULL, LC, LQ, D = 32, 1024, 128, 256
N_CORES = 8
NB = B_FULL // N_CORES  # batches per core
KC = LC // 128  # c-tiles per batch (8)

_CACHE = {}


def _build_nc():
    import concourse.bacc as bacc
    import concourse.mybir as mybir
    from concourse import tile
    from concourse.masks import make_identity

    fp32 = mybir.dt.float32
    fp32r = mybir.dt.float32r
    mmdt = fp32r
    MULT = mybir.AluOpType.mult
    ADD = mybir.AluOpType.add
    EXP = mybir.ActivationFunctionType.Exp

    nc = bacc.Bacc("TRN2", target_bir_lowering=False, debug=False)

    C_d = nc.dram_tensor("C", [NB, LC, D], fp32, kind="ExternalInput")
    qt3_d = nc.dram_tensor("QT3a", [NB, 128, 2, 128], fp32r, kind="ExternalInput")
    qr_d = nc.dram_tensor("Qr", [NB, 128, D + 2], fp32r, kind="ExternalInput")
    cb_d = nc.dram_tensor("cb", [NB, 128, KC], fp32, kind="ExternalInput")
    qb_d = nc.dram_tensor("qb", [NB, 128, 1], fp32, kind="ExternalInput")
    out_d = nc.dram_tensor("out", [NB, LC, 4 * D], fp32, kind="ExternalOutput")

    with tile.TileContext(nc) as tc:
        with (
            tc.tile_pool(name="const", bufs=1) as const,
            tc.tile_pool(name="crpool", bufs=2) as p_cr,
            tc.tile_pool(name="qpool", bufs=NB) as p_q,
            tc.tile_pool(name="mpool", bufs=NB) as p_m,
            tc.tile_pool(name="ctpool", bufs=3) as p_ct,
            tc.tile_pool(name="qtpool", bufs=2) as p_qt,
            tc.tile_pool(name="epool", bufs=3) as p_e,
            tc.tile_pool(name="xgpool", bufs=3) as p_xg,
            tc.tile_pool(name="opool", bufs=12) as p_o,
            tc.tile_pool(name="smpool", bufs=4) as p_sm,
            tc.tile_pool(name="flex", bufs=2, space="PSUM") as ps_flex,
            tc.tile_pool(name="psstt", bufs=2, space="PSUM") as ps_stt,
            tc.tile_pool(name="pst1", bufs=1, space="PSUM") as ps_t1,
            tc.tile_pool(name="psab", bufs=3, space="PSUM") as ps_ab,
        ):
            ident = const.tile([128, 128], fp32)
            make_identity(nc, ident)
            # warm up the ACT exp table while input DMAs run
            wtmp = const.tile([128, 1], fp32, tag="wtmp")
            nc.scalar.activation(wtmp, ident[:, 0:1], EXP)

            # C1r slots: fp32r copies of C for the T1 matmul moving operand.
            # The two ones-pad columns are written once per slot and persist.
            cr_slots = []
            for _ in range(2):
                cr = p_cr.tile([128, KC, D + 2], mmdt, tag="cr")
                nc.vector.memset(cr.bitcast(fp32)[:, :, D : D + 2], 1.0)
                cr_slots.append(cr)

            # ---- hoisted input loads for all batches ----
            osbs, Q1s, cnegs, qnegs = [], [], [], []
            for b in range(NB):
                # output accumulation tiles [C | A | C*A | C*Bm] per half-batch;
                # C lands in cols 0:D straight from HBM (c = 512h+4p+j mapping).
                osb = []
                for g in range(4):
                    o = p_o.tile([128, 2, 4 * D], fp32, tag="osb")
                    nc.sync.dma_start(
                        o[:, :, 0:D],
                        C_d.ap()[b, g * 256 : (g + 1) * 256].rearrange(
                            "(p j) d -> p j d", p=128
                        ),
                    )
                    osb.append(o)
                QT3 = p_q.tile([128, 2, 128], mmdt, tag="qt3")
                nc.sync.dma_start(QT3, qt3_d.ap()[b])
                Q1r = p_q.tile([128, D + 2], mmdt, tag="qr")
                nc.sync.dma_start(Q1r, qr_d.ap()[b])
                cb = p_m.tile([128, KC], fp32, tag="cb")
                nc.sync.dma_start(cb, cb_d.ap()[b])
                qb = p_m.tile([128, 1], fp32, tag="qb")
                nc.sync.dma_start(qb, qb_d.ap()[b])
                osbs.append(osb)
                Q1s.append((QT3, Q1r))
                cnegs.append(cb)
                qnegs.append(qb)

            for b in range(NB):
                osb, (QT3, Q1r), cb, bias_q = osbs[b], Q1s[b], cnegs[b], qnegs[b]

                def Cb(t):
                    return osb[t // 2][:, t % 2, 0:D]

                # rounded fp32r copies for matmul moving operands
                C1r = cr_slots[b % 2]
                for g in range(4):
                    nc.vector.tensor_copy(
                        C1r[:, g * 2 : (g + 1) * 2, 0:D], osb[g][:, :, 0:D]
                    )

                # ---- CT (transpose C): 4 transposes per PSUM bank, 1 copy ----
                CT = p_ct.tile([128, 2, LC], mmdt, tag="ct")
                for dk in range(2):
                    for h in range(2):
                        pt = ps_flex.tile([128, 512], fp32, tag="flex")
                        for jj in range(4):
                            j = h * 4 + jj
                            nc.tensor.transpose(
                                pt[:, jj * 128 : (jj + 1) * 128],
                                Cb(j)[:, dk * 128 : (dk + 1) * 128],
                                ident,
                            )
                        dst = CT[:, dk, h * 512 : (h + 1) * 512]
                        nc.vector.tensor_copy(dst, pt)

                # ---- ST = (Q*w3) @ C^T, E_q = exp(ST + bias_q) ----
                E_q = p_e.tile([128, LC], mmdt, tag="eq")
                for h in range(2):
                    st = ps_flex.tile([128, 512], fp32, tag="flex")
                    for dk in range(2):
                        nc.tensor.matmul(
                            st,
                            QT3[:, dk],
                            CT[:, dk, h * 512 : (h + 1) * 512],
                            start=(dk == 0),
                            stop=(dk == 1),
                        )
                    nc.scalar.activation(
                        E_q[:, h * 512 : (h + 1) * 512], st, EXP, bias=bias_q
                    )

                # ---- STT tiles: [c, q | rc] ; Xg = exp(STT + rc + cneg) ----
                Xg = p_xg.tile([128, KC, 128], mmdt, tag="xg")
                for j in range(KC):
                    stt = ps_stt.tile([128, 128], fp32, tag="stt")
                    for dk in range(2):
                        nc.tensor.matmul(
                            stt,
                            CT[:, dk, j * 128 : (j + 1) * 128],
                            QT3[:, dk],
                            start=(dk == 0),
                            stop=(dk == 1),
                        )
                    nc.scalar.activation(
                        Xg[:, j], stt, EXP, bias=cb[:, j : j + 1]
                    )

                # ---- T1s = (Sc^T C) = (Xg^T @ [C|1]) normalized ----
                t1 = ps_t1.tile([128, D + 2], fp32, tag="t1")
                for j in range(KC):
                    nc.tensor.matmul(
                        t1,
                        Xg[:, j],
                        C1r[:, j],
                        start=(j == 0),
                        stop=(j == KC - 1),
                    )
                recipT = p_sm.tile([128, 1], fp32, tag="recipT")
                nc.vector.reciprocal(recipT, t1[:, D : D + 1])
                T1s = p_sm.tile([128, D], mmdt, tag="t1s")
                nc.vector.tensor_scalar_mul(T1s, t1[:, 0:D], recipT)

                # ---- per c-tile: A / CA / CBm into osb segments ----
                for t in range(KC):
                    h, j = t // 2, t % 2
                    o = osb[h]
                    eq_j = E_q[:, t * 128 : (t + 1) * 128]
                    psA = ps_ab.tile([128, D + 2], fp32, tag="ab")
                    nc.tensor.matmul(psA, eq_j, Q1r[:], start=True, stop=True)
                    psBt = ps_ab.tile([128, D + 2], fp32, tag="ab")
                    psB = psBt[:, 0:D]
                    nc.tensor.matmul(psB, eq_j, T1s[:], start=True, stop=True)

                    rr = p_sm.tile([128, 1], fp32, tag="rr")
                    nc.vector.reciprocal(rr, psA[:, D : D + 1])

                    # A = psA * rr  (ACT, per-partition scale)
                    nc.scalar.mul(o[:, j, D : 2 * D], psA[:, 0:D], rr)
                    # CA = C * A  (GPSIMD, reads the extracted A)
                    nc.gpsimd.tensor_mul(
                        o[:, j, 2 * D : 3 * D],
                        o[:, j, 0:D],
                        o[:, j, D : 2 * D],
                    )
                    # CBm = (psB * rr) * C  (DVE fused)
                    nc.vector.scalar_tensor_tensor(
                        o[:, j, 3 * D : 4 * D],
                        psB,
                        rr,
                        o[:, j, 0:D],
                        MULT,
                        MULT,
                    )
                    # store each quarter as soon as its 2 c-tiles are done:
                    # 1MB DMA, 8KB contiguous per partition
                    if j == 1:
                        nc.sync.dma_start(
                            out_d.ap()[b, h * 256 : (h + 1) * 256].rearrange(
                                "(p j) n -> p j n", p=128
                            ),
                            osb[h],
                        )

    nc.compile()
    return nc


def _get_nc():
    if "nc" not in _CACHE:
        _CACHE["nc"] = _build_nc()
    return _CACHE["nc"]


def _make_in_maps(C, Q, cmask, qmask, Wo_w):
    """Host preprocessing: biases (rq+qneg, rc+cneg), Q^T*w3, [Q|1] fp32r."""
    C = np.ascontiguousarray(C, dtype=np.float32)
    Q = np.ascontiguousarray(Q, dtype=np.float32)
    w1 = Wo_w[0:D].astype(np.float32)
    w2 = Wo_w[D : 2 * D].astype(np.float32)
    w3 = Wo_w[2 * D :].astype(np.float32)
    rc = C @ w1  # [B, LC]
    rq = Q @ w2  # [B, LQ]
    cneg = rc + (1.0 - cmask.astype(np.float32)) * NEG_INF
    qneg = rq + (1.0 - qmask.astype(np.float32)) * NEG_INF
    cneg = cneg.astype(np.float32)
    qneg = qneg.astype(np.float32)
    # c = 256g + 2p + j mapping -> [b, p, (g j)]
    cneg = np.ascontiguousarray(
        cneg.reshape(B_FULL, 4, 128, 2).transpose(0, 2, 1, 3).reshape(B_FULL, 128, KC)
    )
    qneg = np.ascontiguousarray(qneg.reshape(B_FULL, 128, 1))
    # QT3a[b, p, dk, q] = Q[b, q, dk*128+p] * w3[dk*128+p]
    QT3a = np.ascontiguousarray(
        (Q * w3).transpose(0, 2, 1).reshape(B_FULL, 2, 128, LQ).transpose(0, 2, 1, 3)
    )
    Qr = np.ones((B_FULL, 128, D + 2), dtype=np.float32)
    Qr[:, :, 0:D] = Q
    in_maps = []
    for i in range(N_CORES):
        sl = slice(i * NB, (i + 1) * NB)
        in_maps.append(
            {
                "C": np.ascontiguousarray(C[sl]),
                "QT3a": np.ascontiguousarray(QT3a[sl]),
                "Qr": np.ascontiguousarray(Qr[sl]),
                "cb": np.ascontiguousarray(cneg[sl]),
                "qb": np.ascontiguousarray(qneg[sl]),
            }
        )
    return in_maps


def kernel(C, Q, cmask, qmask, Wo_w, Wo_b):
    from concourse.bass_utils import run_bass_kernel_spmd

    nc = _get_nc()
    in_maps = _make_in_maps(C, Q, cmask, qmask, Wo_w)
    res = run_bass_kernel_spmd(nc, in_maps, core_ids=list(range(N_CORES)))
    out = np.concatenate([res.results[i]["out"] for i in range(N_CORES)], axis=0)
    return out
